# revision 1
# baseline (speedup 1.0000x reference)
"""AxialAttention3D Trainium2 Bass kernel (fp8 projections + pipelined attention).

Reference computes, for x [B=2, C=512, D=32, H=32, W=32]:
  qkv = 1x1x1 conv (w_qkv [1536,512]) -> q,k,v [B,512,D,H,W]
  8-head attention along the D axis, independent per (b,h,w,head), hd=64
  out = 1x1x1 conv (w_out) + b_out + x  (residual)

Sharding: 64 (b,h)-slices split across 8 cores (8 slices/core). Each slice is
x[b,:,:,h,:] = [C=512, N=1024 tokens], tokens kept w-major inside the kernel.

Big projections (QK, V^T, out) run in fp8e4m3 with MatmulPerfMode.DoubleRow
(2 K-chunks per instruction, 2x PE throughput). Weights are scaled x64 on the
host before the fp8 cast (w sigma ~0.02 sits in e4m3's subnormal range
unscaled); the scales fold into existing activation scale/bias params:
  q~ = 64(q+bq), k~ = 64k  -> exp scale /= 4096 (k bias cancels in softmax)
  vt~ = 64v -> ao~ = 64*ao -> ao8 = fp8(64*ao)
  out_psum = (64 wout)^T (64 ao) = 4096*out -> out act scale = 1/4096
  v bias commutes through attention -> folded into b_out on host.

Attention (scores/AV) stays bf16 with 32x32 quadrant-packed matmuls
(tile_position); those are moving-column bound (13ns) and don't benefit
from fp8.

Hardware constraint that shapes the schedule: GPSIMD (Pool) cannot read
PSUM, so every PSUM drain lives on Act or DVE; Pool gets only SBUF-side
work (x cast, softmax normalize-mul, residual add).

Schedule per slice (emission order == per-engine program order):
  - x arrives as bf16 (halves input DMA); prefetch distance 2 (xin
    bufs=3); x(s+2) DMA is issued at the TOP of slice s so it never
    queues behind slice-s out stores on the SP queue.
  - slice-level software pipeline: QK/V projections of slice s+1 are
    emitted between attention(s) and out-proj(s), so each engine always
    has a deep well of independent work.
  - attention groups pipelined: scores(g) runs 2 groups ahead of AV(g);
    the P-transpose lags the normalize-mul by one group so the DVE never
    waits on Pool. Softmax row-sum/recip/mul handle both head-parities
    in single merged ops.
  - engine split: Act = exp + q-bias-act + k copy + vt copy;
    DVE = reduce/recip/transpose + AV copies + out-proj drains;
    Pool = x casts + normalize-mul + residual.
  - ao8 is stored in (g,wq)-blocked form so AV copies are contiguous
    [128,128]; the out-proj moving operand reads it with a 3-level
    strided AP.
  - PSUM banks (8 x 2KB): proj pool 2, score pool 2, AV pool 4.
"""

import os
import sys

import numpy as np
import ml_dtypes

sys.path.insert(0, "/opt/trn_rl_repo")

B, C, D, H, W = 2, 512, 32, 32, 32
NH, HD = 8, 64
NCORES = 8
SLICES_PER_CORE = (B * H) // NCORES  # 8
NTOK = D * W  # 1024 tokens per slice

WSCALE = 64.0
EXP_SCALE = float(HD) ** -0.5 / 2 / (WSCALE * WSCALE)
OUT_SCALE = 1.0 / (WSCALE * WSCALE)

LAST_RESULTS = None  # set on each kernel() call; test harness reads exec time


def _build(reps=0):
    """reps=0: straight-line kernel. reps>0: wrap the whole pipeline in a
    hardware For_i loop that recomputes it `reps` times (benchmark only)."""
    import concourse.bass as bass
    from concourse import bacc, mybir
    import concourse.tile as tile
    from contextlib import nullcontext

    bf16 = mybir.dt.bfloat16
    f32 = mybir.dt.float32
    f8 = mybir.dt.float8e4
    Act = mybir.ActivationFunctionType
    DR = mybir.MatmulPerfMode.DoubleRow
    ablate = os.environ.get("KABLATE", "")  # "attn" (bench only)

    nc = bacc.Bacc("TRN2", target_bir_lowering=False, debug=False)

    S = SLICES_PER_CORE
    xs_d = nc.dram_tensor("xs", [S, C, NTOK], bf16, kind="ExternalInput")
    wqkT_d = nc.dram_tensor("wqkT", [C, 2 * C], f8, kind="ExternalInput")
    wvT_d = nc.dram_tensor("wvT", [C, C], f8, kind="ExternalInput")
    woutT_d = nc.dram_tensor("woutT", [C, C], f8, kind="ExternalInput")
    bq_d = nc.dram_tensor("bq", [C], f32, kind="ExternalInput")
    bout_d = nc.dram_tensor("bout", [C], f32, kind="ExternalInput")
    out_d = nc.dram_tensor("out", [S, C, NTOK], f32, kind="ExternalOutput")

    with tile.TileContext(nc) as tc:
        with tc.tile_pool(name="consts", bufs=1) as consts, \
             tc.tile_pool(name="xin", bufs=3) as xin, \
             tc.tile_pool(name="x8p", bufs=2) as x8p, \
             tc.tile_pool(name="qkp", bufs=2) as qkp, \
             tc.tile_pool(name="vtp", bufs=2) as vtp, \
             tc.tile_pool(name="aop", bufs=2) as aop, \
             tc.tile_pool(name="pp", bufs=4) as pp, \
             tc.tile_pool(name="ttp", bufs=10) as ttp, \
             tc.tile_pool(name="smp", bufs=4) as smp, \
             tc.tile_pool(name="outp", bufs=2) as outp, \
             tc.tile_pool(name="psP", bufs=2, space="PSUM") as psP, \
             tc.tile_pool(name="pss", bufs=2, space="PSUM") as pss, \
             tc.tile_pool(name="psav", bufs=4, space="PSUM") as psav:

            # ---- constants (tiles only; DMAs are emitted after x0 so the
            # first slice's input isn't queued behind 1MB of weights) ----
            wqkT_sb = consts.tile([128, 4, 2 * C], f8)   # [c'%128, c'//128, o]
            wvT_sb = consts.tile([128, 4, C], f8)
            woutT_sb = consts.tile([128, 4, C], f8)
            bq_sb = consts.tile([128, 4], f32)   # [o%128, o//128], pre-scaled x64
            bout_sb = consts.tile([128, 4], f32)

            def dma_consts():
                for k in range(4):
                    nc.sync.dma_start(out=wqkT_sb[:, k, :], in_=wqkT_d.ap()[k * 128:(k + 1) * 128, :])
                for k in range(4):
                    nc.sync.dma_start(out=wvT_sb[:, k, :], in_=wvT_d.ap()[k * 128:(k + 1) * 128, :])
                    nc.sync.dma_start(out=woutT_sb[:, k, :], in_=woutT_d.ap()[k * 128:(k + 1) * 128, :])
                nc.gpsimd.dma_start(out=bq_sb, in_=bq_d.ap().rearrange("(t p) -> p t", p=128))
                nc.gpsimd.dma_start(out=bout_sb, in_=bout_d.ap().rearrange("(t p) -> p t", p=128))

            def dma_x(s, x_sb):
                for k in range(4):
                    nc.sync.dma_start(out=x_sb[:, k, :], in_=xs_d.ap()[s, k * 128:(k + 1) * 128, :])

            def cast_x(x_sb, x8):
                # f32 -> fp8 AND permute tokens (d,w) -> w-major (w,d).
                # SBUF->SBUF, so it can live on the otherwise-idle Pool engine
                # (GPSIMD cannot touch PSUM on real HW, so Pool gets only
                # SBUF-side work: casts, softmax mul, residual).
                for k in range(4):
                    nc.gpsimd.tensor_copy(
                        out=x8[:, k, :].rearrange("p (w d) -> p w d", w=32, d=32),
                        in_=x_sb[:, k, :].rearrange("p (d w) -> p w d", d=32, w=32))

            AVLAG = 4  # AV(g) issued AVLAG groups behind scores(g)

            def qk_chunk(x8, qk_sb, t, n2, dve_drain=False):
                # QK projection for channel chunk t, token half n2 (fp8 DoubleRow)
                ps = psP.tile([128, 512], f32, tag="proj", name="ps_qk")
                for n4 in range(2):
                    for kp in range(2):
                        nc.tensor.matmul(
                            ps[:, n4 * 256:(n4 + 1) * 256],
                            wqkT_sb[:, 2 * kp:2 * kp + 2, t * 128:(t + 1) * 128],
                            x8[:, 2 * kp:2 * kp + 2,
                               n2 * 512 + n4 * 256:n2 * 512 + (n4 + 1) * 256],
                            start=(kp == 0), stop=(kp == 1), perf_mode=DR)
                dst = qk_sb[:, t, n2 * 512:(n2 + 1) * 512]
                if t < 4:  # q: apply (scaled) bias
                    if dve_drain:
                        nc.vector.tensor_scalar_add(out=dst, in0=ps,
                                                    scalar1=bq_sb[:, t:t + 1])
                    else:
                        nc.scalar.activation(
                            out=dst, in_=ps,
                            func=Act.Identity, bias=bq_sb[:, t:t + 1], scale=1.0)
                else:      # k: bias cancels in softmax; plain copy
                    if dve_drain:
                        nc.vector.tensor_copy(out=dst, in_=ps)
                    else:
                        nc.scalar.copy(out=dst, in_=ps)

            def v_chunk(x8, vt_sb, g, dve_drain=False):
                # V^T projection group g (w-major tokens on partitions)
                ps = psP.tile([128, 512], f32, tag="proj", name="ps_vt")
                for n in range(2):
                    for kp in range(2):
                        nc.tensor.matmul(
                            ps[:, n * 256:(n + 1) * 256],
                            x8[:, 2 * kp:2 * kp + 2, g * 128:(g + 1) * 128],
                            wvT_sb[:, 2 * kp:2 * kp + 2, n * 256:(n + 1) * 256],
                            start=(kp == 0), stop=(kp == 1), perf_mode=DR)
                if dve_drain:
                    nc.vector.tensor_copy(out=vt_sb[:, g, :], in_=ps)
                else:
                    nc.scalar.copy(out=vt_sb[:, g, :], in_=ps)

            def proj_emitters(x8, qk_sb, vt_sb, prolog=False):
                # 24 projection chunks for a slice, n2-major so the next
                # slice's first attention groups drain first. In the prolog
                # (slice 0) the DVE is idle, so alternate drains between
                # Act and DVE to halve the startup projection phase.
                ems = [(lambda t=t, n2=n2, d=(prolog and (t + n2) % 2 == 1):
                        qk_chunk(x8, qk_sb, t, n2, dve_drain=d))
                       for n2 in range(2) for t in range(8)]
                ems += [(lambda g=g, d=(prolog and g % 2 == 1):
                         v_chunk(x8, vt_sb, g, dve_drain=d)) for g in range(8)]
                return ems

            loop_cm = tc.For_i(0, reps, 1) if reps > 0 else nullcontext()
            with loop_cm:
              x_sb_ring = {}
              x8_ring = {}
              qkv_ring = {}
              x_sb_ring[0] = xin.tile([128, 4, NTOK], bf16, tag="x", name="x_sb")
              dma_x(0, x_sb_ring[0])
              dma_consts()
              x_sb_ring[1] = xin.tile([128, 4, NTOK], bf16, tag="x", name="x_sb")
              dma_x(1, x_sb_ring[1])
              x8_ring[0] = x8p.tile([128, 4, NTOK], f8, tag="x8", name="x8")
              cast_x(x_sb_ring[0], x8_ring[0])
              qkv_ring[0] = (qkp.tile([128, 8, NTOK], bf16, tag="qk", name="qk_sb"),
                             vtp.tile([128, 8, C], bf16, tag="vt", name="vt_sb"))
              for e in proj_emitters(x8_ring[0], *qkv_ring[0], prolog=True):
                  e()
              for s in range(S):
                # prefetch x two slices ahead (before this slice's out stores
                # are queued on SP)
                if s + 2 < S:
                    x_sb_ring[s + 2] = xin.tile([128, 4, NTOK], bf16, tag="x", name="x_sb")
                    dma_x(s + 2, x_sb_ring[s + 2])
                x_sb = x_sb_ring.pop(s)
                x8 = x8_ring.pop(s)
                qk_sb, vt_sb = qkv_ring.pop(s)

                # ---- attention (software pipelined) ----
                # ao8 blocked: [128, (g,wq) block, (q,i)] so AV copies are
                # contiguous [128,128]; out-proj reads per-g strided views
                ao8 = aop.tile([128, 32, 128], f8, tag="ao")
                o_sb = [outp.tile([128, NTOK], f32, tag=f"o{t}", name=f"o_sb{t}")
                        for t in range(4)]
                tdict = {}

                pdict = {}

                def scores_softmax(g):
                    s_ps = [pss.tile([128, 128], f32, tag="s", name=f"s_ps{p}")
                            for p in range(2)]
                    for q in range(4):
                        for wq in range(4):
                            for par in range(2):
                                n = 2 * q + par
                                base = 64 * par
                                toff = (4 * g + wq) * 32
                                qa = qk_sb[base:base + 64, n // 2, toff:toff + 32]
                                ka = qk_sb[base:base + 64, 4 + n // 2, toff:toff + 32]
                                nc.tensor.matmul(
                                    s_ps[par][wq * 32:wq * 32 + 32, q * 32:q * 32 + 32],
                                    qa, ka, start=True, stop=True,
                                    tile_position=(base, wq * 32))
                    p_sb = pp.tile([128, 256], bf16, tag="p", name="p_sb")
                    sums = smp.tile([128, 8], f32, tag="sums", name="sums")
                    for p in range(2):
                        nc.scalar.activation(out=p_sb[:, p * 128:(p + 1) * 128],
                                             in_=s_ps[p],
                                             func=Act.Exp, scale=EXP_SCALE)
                    nc.vector.reduce_sum(
                        out=sums,
                        in_=p_sb.rearrange("p (h j) -> p h j", h=8),
                        axis=mybir.AxisListType.X)
                    nc.vector.reciprocal(out=sums, in_=sums)
                    nc.gpsimd.tensor_mul(
                        out=p_sb.rearrange("p (h j) -> p h j", h=8),
                        in0=p_sb.rearrange("p (h j) -> p h j", h=8),
                        in1=sums.unsqueeze(2).broadcast_to([128, 8, 32]))
                    pdict[g] = p_sb

                def transpose_p(g):
                    # one group behind the mul so the DVE never waits on Pool
                    p_sb = pdict.pop(g)
                    t_sb = [ttp.tile([128, 128], bf16, tag="t", name=f"t_sb{p}")
                            for p in range(2)]
                    for p in range(2):
                        nc.vector.transpose(out=t_sb[p],
                                            in_=p_sb[:, p * 128:(p + 1) * 128])
                    tdict[g] = t_sb

                def av(g):
                    t_sb = tdict.pop(g)
                    avts = [psav.tile([128, 128], f32, tag="av", name=f"av{wq}")
                            for wq in range(4)]
                    for q in range(4):
                        for wq in range(4):
                            for par in range(2):
                                n = 2 * q + par
                                lhsT = vt_sb[wq * 32:wq * 32 + 32, g, n * 64:n * 64 + 64]
                                rhs = t_sb[par][wq * 32:wq * 32 + 32, q * 32:q * 32 + 32]
                                nc.tensor.matmul(
                                    avts[wq][par * 64:par * 64 + 64, q * 32:q * 32 + 32],
                                    lhsT, rhs, start=True, stop=True,
                                    tile_position=(wq * 32, par * 64))
                    # PSUM reads must be Act/DVE; contiguous [128,128] on DVE
                    for wq in range(4):
                        nc.vector.tensor_copy(
                            out=ao8[:, g * 4 + wq, :], in_=avts[wq])

                def out_half(t, n2):
                    # out proj for channel chunk t, token half n2 (w-major)
                    ps = psP.tile([128, 512], f32, tag="proj", name="ps_out")
                    aov = ao8.rearrange("p (g w) (q i) -> p g q w i", g=8, w=4, q=4)
                    for gi in range(4):
                        g = n2 * 4 + gi
                        for kp in range(2):
                            nc.tensor.matmul(
                                ps[:, gi * 128:(gi + 1) * 128],
                                woutT_sb[:, 2 * kp:2 * kp + 2, t * 128:(t + 1) * 128],
                                aov[:, g, 2 * kp:2 * kp + 2, :, :],
                                start=(kp == 0), stop=(kp == 1), perf_mode=DR)
                    dst = o_sb[t][:, n2 * 512:(n2 + 1) * 512]
                    nc.vector.tensor_scalar(
                        out=dst, in0=ps, scalar1=OUT_SCALE,
                        scalar2=bout_sb[:, t:t + 1],
                        op0=mybir.AluOpType.mult, op1=mybir.AluOpType.add)

                if "attn" in ablate:
                    nc.gpsimd.memset(ao8, 0.0)
                    if s + 1 < S:
                        x8_ring[s + 1] = x8p.tile([128, 4, NTOK], f8, tag="x8", name="x8")
                        cast_x(x_sb_ring[s + 1], x8_ring[s + 1])
                else:
                    for g in range(8):
                        scores_softmax(g)
                        if g >= 1:
                            transpose_p(g - 1)
                        if g >= AVLAG:
                            av(g - AVLAG)
                        if g == 4 and s + 1 < S:
                            x8_ring[s + 1] = x8p.tile([128, 4, NTOK], f8, tag="x8", name="x8")
                            cast_x(x_sb_ring[s + 1], x8_ring[s + 1])
                    transpose_p(7)
                    for g in range(8 - AVLAG, 8):
                        av(g)

                # next slice's projections as one block after attention (HW
                # prefers coherent per-phase instruction runs; fine-grained
                # interleaving of fp8 proj matmuls into the bf16 attention
                # stream measured 33% SLOWER on hardware), then this slice's
                # out-projection.
                if s + 1 < S:
                    qkv_ring[s + 1] = (
                        qkp.tile([128, 8, NTOK], bf16, tag="qk", name="qk_sb"),
                        vtp.tile([128, 8, C], bf16, tag="vt", name="vt_sb"))
                    for e in proj_emitters(x8_ring[s + 1], *qkv_ring[s + 1]):
                        e()
                for t in range(4):
                    for n2 in range(2):
                        out_half(t, n2)

                # ---- residual + store (w-major tokens; x viewed strided) ----
                for t in range(4):
                    xv = x_sb[:, t, :].rearrange("p (d w) -> p w d", d=32, w=32)
                    ov = o_sb[t].rearrange("p (w d) -> p w d", w=32, d=32)
                    nc.gpsimd.tensor_add(out=ov, in0=ov, in1=xv)
                    nc.sync.dma_start(out=out_d.ap()[s, t * 128:(t + 1) * 128, :], in_=o_sb[t])

    nc.compile()
    return nc


_NC = None


def kernel(x, w_qkv, b_qkv, w_out, b_out):
    global _NC, LAST_RESULTS
    from concourse import bass_utils

    f8 = ml_dtypes.float8_e4m3
    bf = ml_dtypes.bfloat16
    x = np.asarray(x, dtype=np.float32)
    w_qkv = np.asarray(w_qkv, dtype=np.float32)
    b_qkv = np.asarray(b_qkv, dtype=np.float32)
    w_out = np.asarray(w_out, dtype=np.float32)
    b_out = np.asarray(b_out, dtype=np.float32)

    wqkT = np.ascontiguousarray(w_qkv[:2 * C].T * WSCALE).astype(f8)   # [C, 2C]
    wvT = np.ascontiguousarray(w_qkv[2 * C:].T * WSCALE).astype(f8)    # [C, C]
    woutT = np.ascontiguousarray(w_out.T * WSCALE).astype(f8)          # [C, C]
    bq = np.ascontiguousarray(b_qkv[:C] * WSCALE).astype(np.float32)
    # b_v commutes through attention (rows of softmax sum to 1) -> fold into b_out
    bout_eff = (b_out + w_out @ b_qkv[2 * C:]).astype(np.float32)

    if _NC is None:
        _NC = _build()

    xbf = x.astype(bf)
    in_maps = []
    for cid in range(NCORES):
        xs = np.empty((SLICES_PER_CORE, C, NTOK), dtype=bf)
        for i in range(SLICES_PER_CORE):
            gs = cid * SLICES_PER_CORE + i
            b, h = gs // H, gs % H
            xs[i] = xbf[b, :, :, h, :].reshape(C, NTOK)
        in_maps.append(dict(xs=xs, wqkT=wqkT, wvT=wvT, woutT=woutT,
                            bq=bq, bout=bout_eff))

    res = bass_utils.run_bass_kernel_spmd(
        _NC, in_maps, core_ids=list(range(NCORES)),
        trace=bool(os.environ.get("BASS_TRACE")))
    LAST_RESULTS = res

    out = np.empty((B, C, D, H, W), dtype=np.float32)
    for cid in range(NCORES):
        o = np.asarray(res.results[cid]["out"]).astype(np.float32)
        for i in range(SLICES_PER_CORE):  # [S, C, 1024] w-major bf16 tokens
            gs = cid * SLICES_PER_CORE + i
            b, h = gs // H, gs % H
            out[b, :, :, h, :] = o[i].reshape(C, W, D).transpose(0, 2, 1)
    return out

